# revision 1
# baseline (speedup 1.0000x reference)
"""Trainium2 Bass kernel for nn_DAFCN (motion-prediction DAFCN forward).

Structure exploited (verified vs the reference):
  * The attention branch (wq*/wk* convs, dvb) is dead code: the reference
    computes `combined[:, :, :DCT_N]` which selects only the GCN output.
  * The FFC branch (rfft -> 1x1 conv -> relu -> irfft, first 10 steps) is
    linear-relu-linear and is folded into matmuls (M1, M2, M3).
  * The iDCT + MLP are folded: h = relu(gcn_out @ A1 + ffc10 @ B1 + hb),
    out = h @ W2  with  A1 = (mlp_w1[:, :30] @ idct[:, :10]).T,
    B1 = mlp_w1[:, 30:40].T, W2 = mlp_w2[:10].T, and gc7_w folded into
    W7A = gc7_w @ A1, hb = gc7_b @ A1.
  * The DCT of the GCN input is folded into WG1/A1 on the host
    (E^T @ W), so no on-device DCT matmul is needed.

Sharding: pure data parallelism - 1024 samples / 8 cores = 128 per core,
weights replicated.

Device dataflow per core (all matmuls on PE, bf16 in / f32 PSUM out):
  * GCN state y kept transposed ("layout B"): 4 tiles per 8-sample group,
    y[kc] = [128 part = d-in chunk kc, 384 free = (sample, node)].
  * feature mix: u_j = y[:, j*128:(j+1)*128].T @ W  - three 128-row chunks
    (full PE array) accumulated over kc in PSUM, copied+cast to SBUF V_j.
  * node mix: z^T accumulated per V-chunk with host-expanded block-diagonal
    att^T constants (all operands base partition 0, K=128), with
    column-sliced start=True writes to handle the sample straddling chunk
    boundaries.
  * tanh+bias per d-chunk on ScalarE; per-chunk residuals on DVE/Pool so
    the next layer's first matmul unblocks as soon as chunk 0 is done.
"""

import numpy as np

import concourse.bass as bass
from concourse import mybir
from concourse.tile import TileContext

F32 = mybir.dt.float32
BF16 = mybir.dt.bfloat16
AF = mybir.ActivationFunctionType

N_CORES = 8
B_TOT, T_IN, F_FEAT = 1024, 50, 48
SPC = B_TOT // N_CORES          # samples per core
D = 512
DCT_N = 10


# --------------------------------------------------------------------------
# host-side constant folding
# --------------------------------------------------------------------------

def _expand_att(attT):
    """Expanded node-mix rhs for the three 128-row V-chunks of one 8-sample
    group, (s, n) rows = sample-major, 48 nodes each.

    Returns [128, 480]: cols 0:144 = NV0 (out cols 0:144 via two slices
    0:96 / 96:144), 144:336 = NV1 (out cols 96:288), 336:432 = NV2a (out
    288:384), 432:480 = NV2b (out 240:288).
    """
    A = attT  # A[m, n] = att[n, m]
    out = np.zeros((128, 480))
    # NV0: rows = chunk0 = s0(48) s1(48) s2 m0:32
    out[0:48, 0:48] = A
    out[48:96, 48:96] = A
    out[96:128, 96:144] = A[0:32]
    # NV1: rows = chunk1 = s2 m32:48, s3, s4, s5 m0:16 ; cols rel to 96
    out[0:16, 144 + 0:144 + 48] = A[32:48]
    out[16:64, 144 + 48:144 + 96] = A
    out[64:112, 144 + 96:144 + 144] = A
    out[112:128, 144 + 144:144 + 192] = A[0:16]
    # NV2a: rows = chunk2 = s5 m16:48, s6, s7 ; cols 288:384 -> s6, s7
    out[32:80, 336:384] = A
    out[80:128, 384:432] = A
    # NV2b: out cols 240:288 -> s5 tail
    out[0:32, 432:480] = A[16:48]
    return out


def _build_host_consts(inp):
    f8 = np.float64
    w1 = np.asarray(inp["mlp_w1"], f8)       # [256, 40]
    w2 = np.asarray(inp["mlp_w2"], f8)       # [40, 256]
    wg = np.asarray(inp["ffc_wg"], f8)       # [6, 6]
    wl = np.asarray(inp["ffc_wl"], f8)       # [3, 3]
    gc1_w = np.asarray(inp["gc1_w"], f8)     # [10, 512]
    gc1_b = np.asarray(inp["gc1_b"], f8)     # [512]
    gcb_w = np.asarray(inp["gcb_w"], f8)     # [2, 2, 512, 512]
    gcb_b = np.asarray(inp["gcb_b"], f8)     # [2, 2, 512]
    gc7_w = np.asarray(inp["gc7_w"], f8)     # [512, 10]
    gc7_b = np.asarray(inp["gc7_b"], f8)     # [10]
    att1 = np.asarray(inp["gc1_att"], f8)    # [48, 48]
    attb = np.asarray(inp["gcb_att"], f8)    # [2, 2, 48, 48]
    att7 = np.asarray(inp["gc7_att"], f8)    # [48, 48]

    # DCT pair (block length 30)
    N = 30
    kk = np.arange(N)[:, None]
    ii = np.arange(N)[None, :]
    w = np.full((N, 1), np.sqrt(2.0 / N))
    w[0, 0] = np.sqrt(1.0 / N)
    d = w * np.cos(np.pi * (ii + 0.5) * kk / N)
    idct = np.linalg.inv(d)
    dct10 = d[:DCT_N]                        # [10, 30]

    # E: dct_in^T[d, f] = sum_j E[d, j] * seq[40+j, f]
    E = dct10[:, :10].copy()
    E[:, 9] += dct10[:, 10:].sum(axis=1)
    ET = E.T                                 # [10 k, 10 d]

    # MLP folds
    A1 = (w1[:, :30] @ idct[:, :10]).T       # [10, 256]
    B1 = w1[:, 30:40].T                      # [10, 256]
    W7A = gc7_w @ A1                         # [512, 256]
    hb = gc7_b @ A1                          # [256]
    W2 = w2[:10].T                           # [256, 10]

    # FFC fold: rfft / channel mix / (relu) / irfft+local, first 10 steps
    Fm = np.fft.rfft(np.eye(60), axis=-1)    # [60, 31]
    Fr, Fi = Fm.real.T, Fm.imag.T            # [31, 60]
    M1 = (np.einsum("oc,kt->ctok", wg[:, :3], Fr)
          + np.einsum("oc,kt->ctok", wg[:, 3:], Fi)).reshape(3, 60, 186)
    M1f = np.concatenate(
        [M1[:, :49], M1[:, 49:].sum(axis=1, keepdims=True)], axis=1
    ).reshape(150, 186)                      # [(c,t<50), (o,k)]
    Gr = np.fft.irfft(np.eye(31), n=60, axis=-1)[:, :10]
    Gi = np.fft.irfft(1j * np.eye(31), n=60, axis=-1)[:, :10]
    M2 = np.zeros((6, 31, 3, 10))
    for o3 in range(3):
        M2[o3, :, o3, :] = Gr
        M2[o3 + 3, :, o3, :] = Gi
    M2 = M2.reshape(186, 30)                 # [(o,k), (o3,t')]
    M3 = np.einsum("oc,tu->ctou", wl, np.eye(10)).reshape(30, 30)

    import ml_dtypes
    c = {}
    f4 = lambda a: np.ascontiguousarray(a, ml_dtypes.bfloat16)

    # WGCB [128, 16, 512]: (layer l, k-chunk kc) -> w_l[kc*128+p, j]
    WGCB = np.zeros((128, 16, 512))
    for layer in range(4):
        s, ll = divmod(layer, 2)
        wl_ = gcb_w[s, ll]
        for kc in range(4):
            WGCB[:, layer * 4 + kc, :] = wl_[kc * 128:(kc + 1) * 128]
    c["WGCB"] = f4(WGCB)

    W7At = np.zeros((128, 4, 256))
    for kc in range(4):
        W7At[:, kc, :] = W7A[kc * 128:(kc + 1) * 128]
    c["W7AT"] = f4(W7At)

    # expanded node-mix attention constants, 6 layers
    atts = [att1, attb[0, 0], attb[0, 1], attb[1, 0], attb[1, 1], att7]
    ATTN = np.zeros((128, 6, 480))
    for i, a in enumerate(atts):
        ATTN[:, i, :] = _expand_att(a.T)
    c["ATTN"] = f4(ATTN)

    # all small constants packed into one tensor -> one startup DMA
    M2P = np.zeros((128, 2, 96))
    M3P = np.zeros((30, 96))
    for o3 in range(3):
        M2P[:, 0, o3 * 32:o3 * 32 + 10] = M2[:128].reshape(128, 3, 10)[:, o3]
        M2P[:58, 1, o3 * 32:o3 * 32 + 10] = M2[128:].reshape(58, 3, 10)[:, o3]
        M3P[:, o3 * 32:o3 * 32 + 10] = M3.reshape(30, 3, 10)[:, o3]
    BIAS = np.zeros((128, 22))
    tanh_biases = [gc1_b, gcb_b[0, 0], gcb_b[0, 1], gcb_b[1, 0], gcb_b[1, 1]]
    for li, b in enumerate(tanh_biases):
        for mc in range(4):
            BIAS[:, li * 4 + mc] = b[mc * 128:(mc + 1) * 128]
    for mc in range(2):
        BIAS[:, 20 + mc] = hb[mc * 128:(mc + 1) * 128]

    PACK = np.zeros((128, PACK_COLS))
    def put(name, arr):
        c0, c1, rows = PCOL[name]
        assert arr.shape == (rows, c1 - c0), (name, arr.shape)
        PACK[:rows, c0:c1] = arr
    tc_perm = np.array([c * 50 + t for t in range(50) for c in range(3)])
    M1TC = M1f[tc_perm]
    put("M1A", M1TC[:126])
    put("M1B", M1TC[126:150])
    put("WG1F", ET @ gc1_w)
    put("BIAS", BIAS)
    put("M2P0", M2P[:, 0, :])
    put("M2P1", M2P[:58, 1, :])
    tc10 = np.array([c * 10 + t for t in range(10) for c in range(3)])
    put("M3P", M3P[tc10])
    A1FB = np.concatenate([ET @ A1, B1], axis=0)   # [20, 256]
    put("A1FB", A1FB)
    put("W2T0", W2[0:128, :])
    put("W2T1", W2[128:256, :])
    c["PACK"] = f4(PACK)
    c["HB32"] = np.ascontiguousarray(BIAS[:, 20:22], np.float32)
    return c


# packed-constant column map: name -> (col0, col1, rows)
PCOL = {
    "M1A": (0, 186, 126),
    "M1B": (186, 372, 24),
    "WG1F": (372, 884, 10),
    "BIAS": (884, 906, 128),
    "M2P0": (906, 1002, 128),
    "M2P1": (1002, 1098, 58),
    "M3P": (1098, 1194, 30),
    "A1FB": (1194, 1450, 20),
    "W2T0": (1450, 1460, 128),
    "W2T1": (1460, 1470, 128),
}
PACK_COLS = 1470


CONST_SPECS = {
    "PACK": ((128, PACK_COLS), BF16),
    "HB32": ((128, 2), F32),
    "ATTN": ((128, 6, 480), BF16),
    "WGCB": ((128, 16, 512), BF16),
    "W7AT": ((128, 4, 256), BF16),
}


# --------------------------------------------------------------------------
# bass program
# --------------------------------------------------------------------------

def _split_matmul_waits(raw):
    """TRN2 walrus codegen allows only one sync-wait on Matmult/Ldweights.

    Move extra waits onto EventSemaphore instructions inserted just before
    (same engine, in-order execution => semantics preserved).
    """
    import json as _json
    bir = _json.loads(raw)
    for fn in bir["functions"]:
        for bb in fn["blocks"]:
            out = []
            for inst in bb["instructions"]:
                si = inst.get("sync_info")
                if (inst.get("opcode") != "EventSemaphore"
                        and si and len(si.get("on_wait") or []) > 1):
                    waits = si["on_wait"]
                    keep, extras = waits[-1], waits[:-1]
                    ip = len(out)
                    if (inst["opcode"] == "Matmult" and out
                            and out[-1].get("opcode") == "Ldweights"
                            and out[-1].get("engine") == inst["engine"]
                            and not (out[-1].get("sync_info") or {}).get(
                                "on_wait")):
                        ip = len(out) - 1
                    for j, w in enumerate(extras):
                        out.insert(ip + j, {
                            "debug": inst.get("debug", 0),
                            "engine": inst["engine"],
                            "ins": [], "outs": [],
                            "name": f"{inst['name']}_ws{j}",
                            "opcode": "EventSemaphore",
                            "sync_info": {"on_update": [], "on_wait": [w]},
                        })
                    si["on_wait"] = [keep]
                out.append(inst)
            bb["instructions"] = out
    return _json.dumps(bir).encode()


# engine assignment knobs (tuned via the cost model):
#   V_ENG[j]: engine for the j-th PSUM->SBUF V copy ("act" or "dve")
#   RES_ENG[mc]: engine for the residual add ("dve" or "pool")
V_ENG = ("dve", "dve", "act")
V7_ENG = ("dve", "dve", "act")
RES_ENG = ("dve", "dve", "dve", "dve")
HSB_ENG = "act"


def build_nc(spc=SPC):
    """Build the per-core Bass program for `spc` samples (multiple of 16)."""
    assert spc % 16 == 0
    n_sg = spc // 16
    nc = bass.Bass()

    xh = nc.declare_dram_parameter("xseq", [spc, T_IN, F_FEAT], BF16,
                                   isOutput=False)
    ch = {
        name: nc.declare_dram_parameter(name, list(shape), dt_, isOutput=False)
        for name, (shape, dt_) in CONST_SPECS.items()
    }
    oh = nc.declare_dram_parameter("out", [spc, DCT_N, 1, F_FEAT], F32,
                                   isOutput=True)

    with TileContext(nc) as tc:
        with (
            tc.tile_pool(name="consts", bufs=1) as consts,
            tc.tile_pool(name="seq", bufs=2) as p_seq,
            tc.tile_pool(name="zsb", bufs=2) as p_zsb,
            tc.tile_pool(name="ffc", bufs=2) as p_ffc,
            tc.tile_pool(name="vsb", bufs=6) as p_v,
            tc.tile_pool(name="ysb", bufs=10) as p_y,
            tc.tile_pool(name="hsb", bufs=10) as p_h,
            tc.tile_pool(name="hbig", bufs=2) as p_hbig,
            tc.tile_pool(name="osb", bufs=2) as p_osb,
            tc.tile_pool(name="ps_u", bufs=4, space="PSUM") as ps_u,
            tc.tile_pool(name="ps_zt", bufs=4, space="PSUM") as ps_zt,
        ):
            # ---- tiles + per-group input loader ----
            W = {
                name: consts.tile(list(shape), dt_, tag=name, name=name)
                for name, (shape, dt_) in CONST_SPECS.items()
            }

            def PK(name, cs=None):
                """Packed-constant AP: full rows, optional extra col slice."""
                c0, c1, rows = PCOL[name]
                if cs is not None:
                    c0, c1 = c0 + cs[0], min(c1, c0 + cs[1])
                return W["PACK"][0:rows, c0:c1]

            def load_inputs(g0):
                # 16 samples, (t,c)-major rows so each tile is one DMA:
                # XA rows (t 0:42, c), XB rows (t 42:50, c), X10 (t' 0:10, c)
                XA = p_seq.tile([126, 16, 16], BF16, tag="XA")
                XB = p_seq.tile([24, 16, 16], BF16, tag="XB")
                X10 = p_seq.tile([30, 16, 16], BF16, tag="X10")
                # rows 40:50, flat free (s, f) for the GCN input
                seq40 = p_seq.tile([10, 768], BF16, tag="seq40")
                nc.sync.dma_start(
                    out=XA[...],
                    in_=xh[g0:g0 + 16, 0:42].rearrange(
                        "b t (c f) -> (t c) b f", c=3),
                )
                nc.sync.dma_start(
                    out=XB[...],
                    in_=xh[g0:g0 + 16, 42:50].rearrange(
                        "b t (c f) -> (t c) b f", c=3),
                )
                nc.sync.dma_start(
                    out=X10[...],
                    in_=xh[g0:g0 + 16, 0:10].rearrange(
                        "b t (c f) -> (t c) b f", c=3),
                )
                nc.sync.dma_start(
                    out=seq40[...].rearrange("p (b f) -> p b f", f=F_FEAT),
                    in_=xh[g0:g0 + 16, 40:50].rearrange("b t f -> t b f"),
                )
                # second copy of seq40 stacked over the ffc result so the
                # gc7 A1/B1 terms fold into one K=20 matmul (kept separate
                # from seq40 so gc1 does not dep on the ffc DMAs)
                s40b = p_seq.tile([20, 16, 48], BF16, tag="s40b")
                nc.sync.dma_start(
                    out=s40b[0:10, :, :],
                    in_=xh[g0:g0 + 16, 40:50].rearrange("b t f -> t b f"),
                )
                return XA, XB, X10, seq40, s40b

            # ---- DMA issue order: PACK, sg0 inputs, then consts ----
            nc.sync.dma_start(out=W["PACK"][...], in_=ch["PACK"][...])
            inputs0 = load_inputs(0)
            nc.sync.dma_start(out=W["ATTN"][:, 0:2, :],
                              in_=ch["ATTN"][:, 0:2, :])
            for l in range(2):
                nc.sync.dma_start(out=W["WGCB"][:, l * 4:(l + 1) * 4, :],
                                  in_=ch["WGCB"][:, l * 4:(l + 1) * 4, :])
            nc.sync.dma_start(out=W["ATTN"][:, 2:6, :],
                              in_=ch["ATTN"][:, 2:6, :])
            for l in range(2, 4):
                nc.sync.dma_start(out=W["WGCB"][:, l * 4:(l + 1) * 4, :],
                                  in_=ch["WGCB"][:, l * 4:(l + 1) * 4, :])
            nc.sync.dma_start(out=W["W7AT"][...], in_=ch["W7AT"][...])
            nc.sync.dma_start(out=W["HB32"][...], in_=ch["HB32"][...])

            def mm(out, lhsT, rhs, start=True, stop=True):
                nc.tensor.matmul(out=out, lhsT=lhsT, rhs=rhs,
                                 start=start, stop=stop)

            def node_mix(zt, V, lidx, mc, accum=False, stop_last=True):
                """z^T[d-chunk mc, (s,n) 384] += blockdiag-att mix of V chunks.

                zt: PSUM [128, 384] slice target; V: [V0, V1, V2] SBUF tiles
                [128, 512(or 256)]; lidx: ATTN layer index; mc: d-out chunk.
                If accum, all matmuls accumulate (no start=True writes).
                """
                AT = W["ATTN"]
                c0 = mc * 128
                # One start=True marks the whole PSUM bank pending-zero;
                # later start=False writes to untouched bytes take the
                # "first write wins the zero" path, to touched bytes they
                # accumulate - exactly the semantics needed here.
                mm(zt[:, 96:288], V[1][:, c0:c0 + 128],
                   AT[:, lidx, 144:336], start=(not accum), stop=False)
                mm(zt[:, 0:96], V[0][:, c0:c0 + 128],
                   AT[:, lidx, 0:96], start=False, stop=False)
                mm(zt[:, 96:144], V[0][:, c0:c0 + 128],
                   AT[:, lidx, 96:144], start=False, stop=False)
                mm(zt[:, 288:384], V[2][:, c0:c0 + 128],
                   AT[:, lidx, 336:432], start=False, stop=False)
                mm(zt[:, 240:288], V[2][:, c0:c0 + 128],
                   AT[:, lidx, 432:480], start=False, stop=stop_last)

            for sg in range(n_sg):
                g0 = sg * 16
                XA, XB, X10, seq40, s40b = (inputs0 if sg == 0
                                             else load_inputs(g0))

                # ---- FFC stage 1: Z = relu(M1f^T @ X^T) ----
                zp = ps_u.tile([128, 2, 256], F32, tag="u", name="zp")
                for m0, msz, sl in ((0, 128, 0), (128, 58, 1)):
                    mm(zp[0:msz, sl, :], PK("M1A", (m0, m0 + msz)),
                       XA[...], start=True, stop=False)
                    mm(zp[0:msz, sl, :], PK("M1B", (m0, m0 + msz)),
                       XB[...], start=False, stop=True)
                zsb = p_zsb.tile([128, 2, 256], BF16, tag="zsb")
                nc.scalar.activation(zsb[:, 0, :], zp[:, 0, :], AF.Relu)
                nc.scalar.activation(zsb[0:58, 1, :], zp[0:58, 1, :], AF.Relu)

                # ---- FFC stage 2 -> ffc_sb [(o3 pad32, t'), (s, f2)] ----
                fp = ps_u.tile([96, 256], F32, tag="u", name="fp")
                mm(fp[...], PK("M2P0"), zsb[:, 0, :],
                   start=True, stop=False)
                mm(fp[...], PK("M2P1"), zsb[0:58, 1, :],
                   start=False, stop=False)
                mm(fp[...], PK("M3P"), X10[...], start=False, stop=True)
                ffc_s0 = p_ffc.tile([96, 16, 16], BF16, tag="ffc0")
                nc.vector.tensor_copy(ffc_s0[...], fp[...])
                # partition-moving reshuffle (o3: partitions -> free) via
                # SBUF->SBUF DMA so the B1 matmul gets a contiguous out AP;
                # runs ~30us before its gc7 consumer, latency fully hidden.
                for o3 in range(3):
                    nc.sync.dma_start(
                        out=s40b[10:20, :, o3 * 16:(o3 + 1) * 16],
                        in_=ffc_s0[o3 * 32:o3 * 32 + 10, :, :])

                def gcn_layer(g8, src_y, layer, out_pool, out_tag):
                    """One GCN layer for one 8-sample group.

                    src_y: None (gc1: input is seq40) or list of 4 SBUF
                    tiles [128, 384].  Returns list of 4 tanh-output tiles
                    [128, 384] (d-chunk mc on partitions).
                    """
                    Vs = []
                    for j in range(3):
                        u = ps_u.tile([128, 512], F32, tag="u", name="u")
                        if src_y is None:
                            mm(u[...],
                               seq40[:, g8 * 384 + j * 128:
                                     g8 * 384 + (j + 1) * 128],
                               PK("WG1F"))
                        else:
                            for kc in range(4):
                                mm(u[...],
                                   src_y[kc][:, j * 128:(j + 1) * 128],
                                   W["WGCB"][:, (layer - 1) * 4 + kc, :],
                                   start=(kc == 0), stop=(kc == 3))
                        V = p_v.tile([128, 512], BF16, tag="v", name="v")
                        if V_ENG[j] == "act":
                            nc.scalar.copy(V[...], u[...])
                        else:
                            nc.vector.tensor_copy(V[...], u[...])
                        Vs.append(V)
                    outs = []
                    for mc in range(4):
                        zt = ps_zt.tile([128, 384], F32, tag="zt",
                                        name=f"zt{mc}")
                        node_mix(zt, Vs, layer, mc)
                        o = out_pool.tile([128, 384], BF16, tag=out_tag,
                                          name=out_tag)
                        col = layer * 4 + mc
                        nc.scalar.activation(o[...], zt[...], AF.Tanh,
                                             bias=PK("BIAS", (col, col + 1)))
                        outs.append(o)
                    return outs

                # ---- GCN layers, two 8-groups in lockstep ----
                y8s = [gcn_layer(g8, None, 0, p_y, "y8") for g8 in range(2)]
                for st in range(2):
                    has = [gcn_layer(g8, y8s[g8], 1 + st * 2, p_h, "h8")
                           for g8 in range(2)]
                    hbs = [gcn_layer(g8, has[g8], 2 + st * 2, p_h, "h8")
                           for g8 in range(2)]
                    for g8 in range(2):
                        ynew = []
                        for mc in range(4):
                            yt = p_y.tile([128, 384], BF16, tag="y8",
                                          name="ynew")
                            # mc=0 gates the next layer's first matmul:
                            # keep it on DVE (fast 2-input); rest on Pool.
                            if RES_ENG[mc] == "dve":
                                nc.vector.tensor_tensor(
                                    out=yt[...], in0=y8s[g8][mc][...],
                                    in1=hbs[g8][mc][...],
                                    op=mybir.AluOpType.add)
                            else:
                                nc.gpsimd.tensor_add(
                                    yt[...], y8s[g8][mc][...],
                                    hbs[g8][mc][...])
                            ynew.append(yt)
                        y8s[g8] = ynew

                # ---- gc7 + MLP per 8-group ----
                for g8 in range(2):
                    s0 = g8 * 8
                    y8 = y8s[g8]
                    hps = []
                    for mc in range(2):
                        hp = ps_zt.tile([128, 384], F32, tag="zt",
                                        name=f"hp{mc}")
                        # (x @ A1 + ffc10 @ B1)^T in one K=20 matmul
                        # (covers all cols; start=True)
                        mm(hp[...], PK("A1FB", (mc * 128, (mc + 1) * 128)),
                           s40b[:, g8 * 8:(g8 + 1) * 8, :],
                           start=True, stop=False)
                        hps.append(hp)
                    # z7 = (att7 @ (y @ W7A))^T, V-chunked
                    V7s = []
                    for j in range(3):
                        u7 = ps_u.tile([128, 512], F32, tag="u", name="u7")
                        for kc in range(4):
                            mm(u7[:, 0:256],
                               y8[kc][:, j * 128:(j + 1) * 128],
                               W["W7AT"][:, kc, :],
                               start=(kc == 0), stop=(kc == 3))
                        V7 = p_v.tile([128, 512], BF16, tag="v", name="v7")
                        if V7_ENG[j] == "act":
                            nc.scalar.copy(V7[:, 0:256], u7[:, 0:256])
                        else:
                            nc.vector.tensor_copy(V7[:, 0:256], u7[:, 0:256])
                        V7s.append(V7)
                    for mc in range(2):
                        node_mix(hps[mc], V7s, 5, mc, accum=True,
                                 stop_last=True)
                    # relu(h + hb) on DVE
                    hsb = p_hbig.tile([128, 2, 384], BF16, tag="hbig")
                    for mc in range(2):
                        if HSB_ENG == "act":
                            nc.scalar.activation(
                                hsb[:, mc, :], hps[mc][...], AF.Relu,
                                bias=W["HB32"][:, mc:mc + 1])
                        else:
                            nc.vector.tensor_scalar(
                                out=hsb[:, mc, :],
                                in0=hps[mc][...],
                                scalar1=W["HB32"][:, mc:mc + 1],
                                scalar2=0.0,
                                op0=mybir.AluOpType.add,
                                op1=mybir.AluOpType.max)
                    # out = (h @ W2)^T -> [10, (s,f)]
                    op = ps_zt.tile([10, 384], F32, tag="zt", name="op")
                    for mc in range(2):
                        mm(op[...], PK("W2T1" if mc else "W2T0"), hsb[:, mc, :],
                           start=(mc == 0), stop=(mc == 1))
                    osb = p_osb.tile([10, 384], F32, tag="osb")
                    nc.vector.tensor_copy(osb[...], op[...])
                    nc.sync.dma_start(
                        out=oh[g0 + s0:g0 + s0 + 8].rearrange(
                            "b t o f -> t b (o f)"),
                        in_=osb.rearrange("p (s f) -> p s f", f=F_FEAT),
                    )
    _orig_to_json_bytes = nc.to_json_bytes
    nc.to_json_bytes = lambda: _split_matmul_waits(_orig_to_json_bytes())
    return nc


# --------------------------------------------------------------------------
# host entry point
# --------------------------------------------------------------------------

_CACHE = {}


def kernel(**inputs):
    assert int(inputs.get("input_n", 50)) == 50
    assert int(inputs.get("output_n", 20)) == 20
    assert int(inputs.get("itera", 1)) == 1

    import ml_dtypes
    x = np.ascontiguousarray(
        np.asarray(inputs["input_seq"], np.float32).astype(ml_dtypes.bfloat16))
    assert x.shape == (B_TOT, T_IN, F_FEAT)

    consts = _build_host_consts(inputs)

    if "nc" not in _CACHE:
        _CACHE["nc"] = build_nc(SPC)
    nc = _CACHE["nc"]

    from concourse.bass_utils import run_bass_kernel_spmd

    in_maps = []
    for i in range(N_CORES):
        m = dict(consts)
        m["xseq"] = x[i * SPC:(i + 1) * SPC]
        in_maps.append(m)

    res = run_bass_kernel_spmd(nc, in_maps, list(range(N_CORES)))
    out = np.concatenate([res.results[i]["out"] for i in range(N_CORES)],
                         axis=0)
    return out.astype(np.float32)

